# revision 31
# baseline (speedup 1.0000x reference)
"""Biaffine scorer kernel for Trainium2 (Bass/Tile), data-parallel over batch
across 8 NeuronCores. bf16 v4.

Reference computation (per batch item b):
    h = leaky_relu(state @ head_w + head_b)          # (S, BS)
    t = leaky_relu(state @ tail_w + tail_b)          # (S, BS)
    scores1[x,y,o] = h[x] @ U[o] @ t[y]
    scores2[x,y,o] = Wh.h1[x] + Wt.t1[y] + Ww.wemb[x,y] + cls_b
    out = scores1 + scores2                          # (S, S, O)

Device-side decomposition, all bf16 (PSUM fp32), S padded 255->256, batch
items in PAIRS so matmuls stream N=512 moving columns:

    h1T/t1T [128, (bb,x) 512] = Lrelu(head_w.T @ stateT, bias) on ACT.
        Feature rows padded to 128 (120 real + ones-row 120 + zeros); the
        ones-row comes from bias[120]=1 acting on a zero matmul row.
    tu [128, (o, bb, y)] : per o, [U(o).T|Wt+cls_b fold] @ t1T -> one
        contiguous PSUM->SBUF copy per o, alternating ACT/DVE so two
        evacuations stay in flight. The A-term (Wh.h1) and cls_b ride inside
        the ut blocks' ones-row/col.
    out[x, (c,o2,y)] = h1T[:,xtile].T @ tu[:, 2c:2c+2, bb, :]  (N=512),
        pairs of chunks share a 2-bank PSUM tile so evacuation runs at
        FD=1024 (fixed cost amortized), alternating ACT/DVE.

The width-embedding term C[x,y,o] = wproj[pos(x,y), o] is batch-independent
and never touches the device: the HOST adds it during output decode.

Scheduling details: ~24 warmup matmuls on scratch zeros keep the PE HAM
clock-gate warm before the first input-dependent matmul; stateT loads ride
SWDGE (gpsimd) so they never queue behind the busy ACT engine; output DMAs
are split 2048+512 columns so the tail transfer is short.
"""

import numpy as np
import ml_dtypes

import concourse.bass as bass
import concourse.bacc as bacc
import concourse.tile as tile
from concourse import mybir
from concourse.bass_utils import run_bass_kernel_spmd

# problem shape (hardcoded per harness contract)
B, S, H = 32, 255, 1024
BS, WD, O = 120, 20, 10
SP = 256            # padded S
SP2 = 2 * SP        # paired moving dim
KT = H // 128       # 8
NCORES = 8
BPC = B // NCORES   # 4 batch items per core
NP = BPC // 2       # 2 pairs per core
NW = SP * O         # 2560 output cols per (x, b)
NWARM = 6           # PE warmup matmuls

F32 = mybir.dt.float32
BF16 = mybir.dt.bfloat16
NPBF = ml_dtypes.bfloat16

_CACHE: dict = {}


def _emit(tc, d):
    """Emit the per-core program. d: dict of DRAM APs."""
    from contextlib import ExitStack

    nc = tc.nc
    AF = mybir.ActivationFunctionType

    with ExitStack() as ctx:
        const = ctx.enter_context(tc.tile_pool(name="const", bufs=1))
        ht_pool = ctx.enter_context(tc.tile_pool(name="ht", bufs=2))
        tu_pool = ctx.enter_context(tc.tile_pool(name="tu", bufs=2))
        out_pool = ctx.enter_context(tc.tile_pool(name="outp", bufs=4))
        pp_u = ctx.enter_context(tc.tile_pool(name="pp_u", bufs=2, space="PSUM"))
        pp_s = ctx.enter_context(tc.tile_pool(name="pp_s", bufs=3, space="PSUM"))

        # ---- PE warmup: keep HAM at K=8/8 until real matmuls arrive ----
        scratch = const.tile([128, 512], BF16)
        nc.vector.memset(scratch[:], 0.0)
        ps_w = pp_s.tile([128, 1024], F32, tag="ps")
        for wi in range(NWARM):
            nc.tensor.matmul(
                ps_w[:, 0:512],
                lhsT=scratch[:, 0:128],
                rhs=scratch[:],
                start=True,
                stop=True,
            )

        # ---- inputs: 6 DMAs on the sync ring ordered so the projection
        # chain streams right behind the transfers (FIFO == priority):
        # inA = sTa01(p0)|tw01 ; inB = sTa23(p0)|tw2-7 ; inC = sTb(p0) ;
        # inD = hw|tb|hb ; ut ; inE = state(p1) ----
        inA = const.tile([128, 1280], BF16)
        nc.sync.dma_start(inA[:], d["inA"])
        inB = const.tile([128, 1792], BF16)
        nc.sync.dma_start(inB[:], d["inB"])
        inC = const.tile([128, 2048], BF16)
        nc.sync.dma_start(inC[:], d["inC"])
        inD = const.tile([128, 1026], BF16)
        nc.sync.dma_start(inD[:], d["inD"])
        sb_ut = const.tile([128, O * 128], BF16)
        nc.sync.dma_start(sb_ut[:], d["ut"])
        inE = const.tile([128, 4096], BF16)
        nc.sync.dma_start(inE[:], d["inE"])
        tb = inD[:, 1024:1025]
        hb = inD[:, 1025:1026]

        def tw_ap(kt):
            if kt < 2:
                return inA[:, 1024 + kt * 128:1024 + (kt + 1) * 128]
            return inB[:, 1024 + (kt - 2) * 128:1024 + (kt - 1) * 128]

        def hw_ap(kt):
            return inD[:, kt * 128:(kt + 1) * 128]

        def st_ap(p, kt):
            if p == 1:
                return inE[:, kt * 512:(kt + 1) * 512]
            if kt < 2:
                return inA[:, kt * 512:(kt + 1) * 512]
            if kt < 4:
                return inB[:, (kt - 2) * 512:(kt - 1) * 512]
            return inC[:, (kt - 4) * 512:(kt - 3) * 512]

        hts = [None] * NP
        tus = [None] * NP
        t1s = [None] * NP

        def proj_chain(p, ps, w_ap, dst, bv):
            for kt in range(KT):
                nc.tensor.matmul(
                    ps,
                    lhsT=w_ap(kt),
                    rhs=st_ap(p, kt),
                    start=(kt == 0),
                    stop=(kt == KT - 1),
                )
            nc.scalar.activation(dst[:], ps, AF.Lrelu, bias=bv, alpha=0.01)

        def tu_block(p, o_lo, o_hi):
            tu = tus[p]
            t1T = t1s[p]
            for o in range(o_lo, o_hi):
                ps_u = pp_u.tile([128, SP2], F32, tag="ps_u")
                nc.tensor.matmul(
                    ps_u[:],
                    lhsT=sb_ut[:, o * 128:(o + 1) * 128],
                    rhs=t1T[:],
                    start=True,
                    stop=True,
                )
                if o % 2 == 0:
                    nc.scalar.activation(tu[:, o, :, :], ps_u[:], AF.Copy)
                else:
                    nc.vector.tensor_copy(tu[:, o, :, :], ps_u[:])

        _ti = [0]

        def final_tile(p, bb, xt):
            ti = _ti[0]
            _ti[0] += 1
            h1T = hts[p]
            tu = tus[p]
            sb_out = out_pool.tile([128, NW], BF16, tag="sbo")
            lo = bb * SP + xt * 128
            ps_a = pp_s.tile([128, 1024], F32, tag="ps")
            ps_b = pp_s.tile([128, 1024], F32, tag="ps")
            ps_c = pp_u.tile([128, 512], F32, tag="ps_u")
            for c, (dst, off) in enumerate(
                ((ps_a, 0), (ps_a, 512), (ps_b, 0), (ps_b, 512), (ps_c, 0))
            ):
                nc.tensor.matmul(
                    dst[:, off:off + 512],
                    lhsT=h1T[:, lo:lo + 128],
                    rhs=tu[:, 2 * c:2 * c + 2, bb, :],
                    start=True,
                    stop=True,
                )
            # each group evacuated by BOTH engines concurrently (half each)
            # so the PSUM slot frees in ~0.7us instead of ~1.2us
            nc.scalar.activation(sb_out[:, 0:512], ps_a[:, 0:512], AF.Copy)
            nc.vector.tensor_copy(sb_out[:, 512:1024], ps_a[:, 512:1024])
            nc.scalar.activation(sb_out[:, 1024:1536], ps_b[:, 0:512], AF.Copy)
            nc.vector.tensor_copy(sb_out[:, 1536:2048], ps_b[:, 512:1024])
            if ti % 2 == 0:
                nc.scalar.activation(sb_out[:, 2048:2560], ps_c[:], AF.Copy)
            else:
                nc.vector.tensor_copy(sb_out[:, 2048:2560], ps_c[:])
            nc.sync.dma_start(
                d["out"][2 * p + bb, xt, :, 0:1024], sb_out[:, 0:1024]
            )
            nc.sync.dma_start(
                d["out"][2 * p + bb, xt, :, 1024:2560], sb_out[:, 1024:2560]
            )

        # ---- software-pipelined emission: engine FIFOs are in program
        # order, so interleave pair-1 proj/tu into pair-0's finals to keep
        # PE dense while spreading ACT/DVE evacuation load ----
        for p in range(NP):
            hts[p] = ht_pool.tile([128, SP2], BF16, tag="h1T", name=f"h1T_{p}")
            t1s[p] = ht_pool.tile([128, SP2], BF16, tag="t1T", name=f"t1T_{p}")
            tus[p] = tu_pool.tile([128, O, 2, SP], BF16, tag="tu", name=f"tu_{p}")

        ps_p0 = pp_s.tile([128, 1024], F32, tag="ps")
        proj_chain(0, ps_p0[:, 0:512], tw_ap, t1s[0], tb)
        proj_chain(0, ps_p0[:, 512:1024], hw_ap, hts[0], hb)
        tu_block(0, 0, O)

        ps_t1 = pp_s.tile([128, 1024], F32, tag="ps")
        proj_chain(1, ps_t1[:, 0:512], tw_ap, t1s[1], tb)
        final_tile(0, 0, 0)
        ps_h1 = pp_s.tile([128, 1024], F32, tag="ps")
        proj_chain(1, ps_h1[:, 0:512], hw_ap, hts[1], hb)
        final_tile(0, 0, 1)
        tu_block(1, 0, 5)
        final_tile(0, 1, 0)
        tu_block(1, 5, O)
        final_tile(0, 1, 1)
        for bb in range(2):
            for xt in range(2):
                final_tile(1, bb, xt)


def build_nc():
    if "nc" in _CACHE:
        return _CACHE["nc"]
    nc = bacc.Bacc(
        "TRN2", target_bir_lowering=False, debug=False, num_devices=NCORES
    )
    d = {}
    d["inA"] = nc.dram_tensor("inA", [128, 1280], BF16, kind="ExternalInput").ap()
    d["inB"] = nc.dram_tensor("inB", [128, 1792], BF16, kind="ExternalInput").ap()
    d["inC"] = nc.dram_tensor("inC", [128, 2048], BF16, kind="ExternalInput").ap()
    d["inD"] = nc.dram_tensor("inD", [128, 1026], BF16, kind="ExternalInput").ap()
    d["ut"] = nc.dram_tensor("ut", [128, O * 128], BF16, kind="ExternalInput").ap()
    d["inE"] = nc.dram_tensor("inE", [128, 4096], BF16, kind="ExternalInput").ap()
    d["out"] = nc.dram_tensor(
        "out", [BPC, 2, 128, NW], BF16, kind="ExternalOutput"
    ).ap()

    with tile.TileContext(nc) as tc:
        _emit(tc, d)
    nc.compile()
    _CACHE["nc"] = nc
    return nc


def prep_inputs(inputs):
    """Host-side packing + transposes + bf16 conversion. Returns dict of np
    arrays shared across cores (stateT is full-batch; shard before dispatch),
    plus the host-side C addend under key "_C"."""
    state = np.asarray(inputs["state"], np.float32)
    head_w = np.asarray(inputs["head_w"], np.float32)
    head_b = np.asarray(inputs["head_b"], np.float32)
    tail_w = np.asarray(inputs["tail_w"], np.float32)
    tail_b = np.asarray(inputs["tail_b"], np.float32)
    U = np.asarray(inputs["U"], np.float32)
    width_table = np.asarray(inputs["width_table"], np.float32)
    cls_w = np.asarray(inputs["cls_w"], np.float32)
    cls_b = np.asarray(inputs["cls_b"], np.float32)
    BSE = BS + 1

    # stateT paired pack: [B/2, 128, (kt, b01, y)], y zero-padded to 256
    stateT = np.zeros((B, H, SP), NPBF)
    stateT[:, :, :S] = state.transpose(0, 2, 1).astype(NPBF)
    # [B/2, 2, KT, 128, SP] -> [B/2, 128, KT, 2, SP]
    stateT = stateT.reshape(B // 2, 2, KT, 128, SP).transpose(0, 3, 2, 1, 4)
    stateT = np.ascontiguousarray(stateT.reshape(B // 2, 128, KT * SP2))

    # head/tail weights: [128, (kt, j)] with j padded 120->128 (zeros)
    hw_sb = np.zeros((128, KT, 128), np.float32)
    hw_sb[:, :, :BS] = head_w.reshape(KT, 128, BS).transpose(1, 0, 2)
    tw_sb = np.zeros((128, KT, 128), np.float32)
    tw_sb[:, :, :BS] = tail_w.reshape(KT, 128, BS).transpose(1, 0, 2)
    hw_sb = hw_sb.reshape(128, KT * 128).astype(NPBF)
    tw_sb = tw_sb.reshape(128, KT * 128).astype(NPBF)

    # ut blocks [j, (o, i)], j/i padded to 128.
    # block[j, o, i] = U[o, i, j];  col i=120 = Wt_ext[o, j] (B-term);
    # row j=120 += Wh_ext[o, i] (A-term; t1 row 120 == 1);
    # [120, o, 120] += cls_b[o].
    ut = np.zeros((128, O, 128), np.float32)
    ut[:BS, :, :BS] = U.transpose(2, 0, 1)
    ut[:BSE, :, BS] = cls_w[:, BS + 1:2 * BSE].T
    ut[BS, :, :BSE] += cls_w[:, :BSE]
    ut[BS, :, BS] += cls_b
    ut = ut.reshape(128, O * 128).astype(NPBF)

    # biases [128, 2] bf16: col0 tail, col1 head; row 120 = 1.0 (ones feature)
    bias = np.zeros((128, 2), np.float32)
    bias[:BS, 0] = tail_b
    bias[:BS, 1] = head_b
    bias[BS, :] = 1.0
    bias = bias.astype(NPBF)

    # host-side C addend [S, S, O] (width term; wproj[0] = 0 by padding_idx)
    pos = np.arange(S)[None, :] - np.arange(S)[:, None] + 1
    pos = pos * (pos > 0)                                 # [S, S]
    wproj = width_table @ cls_w[:, 2 * BSE:].T            # [256, O]
    cadd = wproj[pos]                                     # [S, S, O] fp32

    return {
        "stateT": stateT,
        "hw": hw_sb,
        "tw": tw_sb,
        "ut": ut,
        "bias": bias,
        "_C": cadd,
    }


def run(inputs, trace=False, trace_kwargs=None):
    nc = build_nc()
    full = prep_inputs(inputs)
    cadd = full.pop("_C")
    stateT = full["stateT"]
    in_maps = []
    tw, hw, bias = full["tw"], full["hw"], full["bias"]
    inD = np.ascontiguousarray(np.concatenate([hw, bias], axis=1))
    for c in range(NCORES):
        p0 = stateT[c * NP]
        p1 = stateT[c * NP + 1]
        m = {
            "inA": np.ascontiguousarray(
                np.concatenate([p0[:, 0:1024], tw[:, 0:256]], axis=1)
            ),
            "inB": np.ascontiguousarray(
                np.concatenate([p0[:, 1024:2048], tw[:, 256:1024]], axis=1)
            ),
            "inC": np.ascontiguousarray(p0[:, 2048:4096]),
            "inD": inD,
            "ut": full["ut"],
            "inE": p1,
        }
        in_maps.append(m)
    res = run_bass_kernel_spmd(
        nc,
        in_maps,
        core_ids=list(range(NCORES)),
        trace=trace,
        **(trace_kwargs or {}),
    )
    out = np.concatenate([r["out"] for r in res.results], axis=0)
    # [B, xt, p, c, o2, y] -> [B, x, y, o]
    out = out.reshape(B, 2, 128, 5, 2, SP).transpose(0, 1, 2, 5, 3, 4)
    out = out.reshape(B, SP, SP, O)[:, :S, :S, :].astype(np.float32)
    out += cadd[None]
    return out, res


def kernel(**inputs):
    out, _ = run(inputs, trace=False)
    return out


if __name__ == "__main__":
    build_nc()
    print("build ok")
